# revision 6
# baseline (speedup 1.0000x reference)
"""Trainium2 Bass kernel for nn_AdaptiveRecurrentModel.

h_{t+1}[b,g] = tanh( (x_t @ Wi.T)[b,g] + sum_h h_t[b,h]*(W_base+ba_r+delta_t[b])[g,h]
                     + (bi+bias)[g] ),   delta_t[b] = (x_t @ Wa.T)[b].reshape(H,H)

Sharding: data-parallel over batch B (8 per core). Weights replicated.

Per-core layout strategy (no transposes anywhere in the T-step recurrence):
 - h kept as hT [h=128 partitions, b=8 free]; the step output tanh(...)[g, b]
   is directly the next step's input layout (g and h both index hidden dim).
 - delta precomputed in blocks of 16 steps (double-buffered) as matmul sweeps:
   for each g: delta[h, g*TB+tb] = WaT[:, g*H:(g+1)*H].T @ tokT_block.
 - per step, one PSUM tile [g, 8] accumulates: Wi matmul (N=8, no h dep,
   issues during previous tanh), W_base matmul (N=8), then 8 per-token
   matvecs with delta slices as stationary weights (N=1, column b).
   ScalarE applies tanh(psum + bias) -> next hT.
 - compute in bf16 (PE fp32 is dual-pass = 2x LDWEIGHTS+MATMUL; the system
   is fp32-chaotic anyway so trajectory-level precision is unattainable).
"""

import sys
from contextlib import ExitStack

import numpy as np

sys.path.insert(0, "/opt/trn_rl_repo")

import ml_dtypes  # noqa: E402
import concourse.bass as bass  # noqa: E402,F401
import concourse.tile as tile  # noqa: E402
from concourse import bacc, mybir  # noqa: E402
from concourse.bass_utils import run_bass_kernel_spmd  # noqa: E402

T_FULL, B, D, H = 256, 64, 128, 128
NCORES = 8
BL = B // NCORES  # 8 batch elements per core
F32 = mybir.dt.float32
BF16 = mybir.dt.bfloat16
BF16_NP = ml_dtypes.bfloat16

_nc_cache: dict = {}


def _make_nc(T: int):
    NTOK = T * BL
    TB = min(128, NTOK)           # tokens per delta block
    assert NTOK % TB == 0
    NBLK = NTOK // TB
    SPB = TB // BL                # steps per block (16)

    nc = bacc.Bacc("TRN2", target_bir_lowering=False, debug=False,
                   num_devices=NCORES)
    tokT_e = nc.dram_tensor("tokT", [D, NTOK], BF16, kind="ExternalInput").ap()
    waT_e = nc.dram_tensor("waT", [D, H * H], BF16, kind="ExternalInput").ap()
    wbT_e = nc.dram_tensor("wbT", [H, H], BF16, kind="ExternalInput").ap()
    wiT_e = nc.dram_tensor("wiT", [D, H], BF16, kind="ExternalInput").ap()
    biasv_e = nc.dram_tensor("biasv", [H, 1], F32, kind="ExternalInput").ap()
    out_e = nc.dram_tensor("out", [H, BL], F32, kind="ExternalOutput").ap()

    with tile.TileContext(nc) as tc:
        with ExitStack() as ctx:
            const = ctx.enter_context(tc.tile_pool(name="const", bufs=1))
            hpool = ctx.enter_context(tc.tile_pool(name="hstate", bufs=2))
            dpool = ctx.enter_context(tc.tile_pool(name="delta", bufs=2))
            pgen = ctx.enter_context(tc.tile_pool(name="pgen", bufs=4,
                                                  space="PSUM"))
            paccp = ctx.enter_context(tc.tile_pool(name="pacc", bufs=2,
                                                   space="PSUM"))

            # Wa^T split into 8 tiles so early delta matmuls can start while
            # later chunks are still streaming from HBM.
            NSPLIT = 8
            GPT = H // NSPLIT     # g's per wa tile (16)
            wa_tiles = []
            for i in range(NSPLIT):
                wt = const.tile([D, GPT * H], BF16, tag=f"wa{i}")
                nc.sync.dma_start(
                    wt[:], waT_e[:, i * GPT * H:(i + 1) * GPT * H])
                wa_tiles.append(wt)
            tokT = const.tile([D, NTOK], BF16, tag="tokT")
            nc.sync.dma_start(tokT[:], tokT_e[:])
            wbT = const.tile([H, H], BF16, tag="wbT")
            nc.sync.dma_start(wbT[:], wbT_e[:])
            wiT = const.tile([D, H], BF16, tag="wiT")
            nc.sync.dma_start(wiT[:], wiT_e[:])
            biasv = const.tile([H, 1], F32, tag="biasv")
            nc.sync.dma_start(biasv[:], biasv_e[:])

            hT = hpool.tile([H, BL], BF16, tag="h")
            nc.vector.memset(hT[:], 0.0)

            GG = 4                                 # g's per PSUM tile
            NCHUNK = H // GG                       # gen chunks per block (32)

            def gen_chunk(delta_t, kblk, g0):
                """Emit GG matmuls + one DVE copy producing
                delta_t[:, g0*TB:(g0+GG)*TB] for block kblk."""
                tok_blk = tokT[:, kblk * TB:(kblk + 1) * TB]
                ps = pgen.tile([H, GG * TB], F32, tag="pgen")
                for gg in range(GG):
                    g = g0 + gg
                    wt = wa_tiles[g // GPT]
                    lhsT = wt[:, (g % GPT) * H:((g % GPT) + 1) * H]
                    nc.tensor.matmul(ps[:, gg * TB:(gg + 1) * TB],
                                     lhsT, tok_blk,
                                     start=True, stop=True,
                                     skip_group_check=True)
                nc.vector.tensor_copy(delta_t[:, g0 * TB:(g0 + GG) * TB],
                                      ps[:])

            # prologue: block 0's delta
            cur = dpool.tile([H, H * TB], BF16, tag="delta")  # [h, g*TB+tb]
            for ci in range(NCHUNK):
                gen_chunk(cur, 0, ci * GG)

            GEN_PER_STEP = (NCHUNK + SPB - 1) // SPB          # 2
            for kblk in range(NBLK):
                if kblk + 1 < NBLK:
                    nxt = dpool.tile([H, H * TB], BF16, tag="delta")
                else:
                    nxt = None
                d_r = cur[:].rearrange("p (g t) -> p g t", t=TB)
                gi = 0
                for s in range(SPB):
                    t = kblk * SPB + s
                    pacc = paccp.tile([H, BL], F32, tag="pacc")
                    nc.tensor.matmul(pacc[:, 0:BL], wiT[:],
                                     tokT[:, t * BL:(t + 1) * BL],
                                     start=True, stop=False,
                                     skip_group_check=True)
                    nc.tensor.matmul(pacc[:, 0:BL], wbT[:], hT[:],
                                     start=False, stop=False,
                                     skip_group_check=True)
                    for b in range(BL):
                        tb = s * BL + b
                        nc.tensor.matmul(pacc[:, b:b + 1],
                                         d_r[:, :, tb:tb + 1],
                                         hT[:, b:b + 1],
                                         start=False, stop=(b == BL - 1),
                                         skip_group_check=True)
                    hT_new = hpool.tile([H, BL], BF16, tag="h")
                    nc.scalar.activation(hT_new[:], pacc[:],
                                         mybir.ActivationFunctionType.Tanh,
                                         bias=biasv[:])
                    # next block's delta generation fills the PE pipeline
                    # while ScalarE runs the tanh this step depends on
                    if nxt is not None:
                        for _ in range(GEN_PER_STEP):
                            if gi < NCHUNK:
                                gen_chunk(nxt, kblk + 1, gi * GG)
                                gi += 1
                    hT = hT_new
                cur = nxt

            fin = const.tile([H, BL], F32, tag="fin")
            nc.vector.tensor_copy(fin[:], hT[:])
            nc.sync.dma_start(out_e[:], fin[:])
    nc.finalize()
    return nc


def _prep_inputs(tokens, Wi, bi, W_base, bias, Wa, ba):
    T = tokens.shape[0]
    Wb_eff = (np.asarray(W_base, np.float32)
              + np.asarray(ba, np.float32).reshape(H, H))
    waT = np.ascontiguousarray(np.asarray(Wa, np.float32).T
                               .astype(BF16_NP))                 # [d, g*H+h]
    wbT = np.ascontiguousarray(Wb_eff.T.astype(BF16_NP))         # [h, g]
    wiT = np.ascontiguousarray(np.asarray(Wi, np.float32).T
                               .astype(BF16_NP))                 # [d, g]
    biasv = np.ascontiguousarray(
        (np.asarray(bi, np.float32) + np.asarray(bias, np.float32))
        .reshape(H, 1))
    in_maps = []
    for c in range(NCORES):
        tok_c = np.asarray(tokens[:, c * BL:(c + 1) * BL, :], np.float32)
        tokT = np.ascontiguousarray(
            tok_c.transpose(2, 0, 1).reshape(D, T * BL).astype(BF16_NP))
        in_maps.append({"tokT": tokT, "waT": waT, "wbT": wbT,
                        "wiT": wiT, "biasv": biasv})
    return in_maps


def _run(T: int, in_maps, trace=False):
    if T not in _nc_cache:
        _nc_cache[T] = _make_nc(T)
    nc = _nc_cache[T]
    res = run_bass_kernel_spmd(nc, in_maps, core_ids=list(range(NCORES)),
                               trace=trace)
    return res


def kernel(tokens, Wi, bi, W_base, bias, Wa, ba):
    tokens = np.asarray(tokens, dtype=np.float32)
    T = tokens.shape[0]
    in_maps = _prep_inputs(tokens, Wi, bi, W_base, bias, Wa, ba)
    res = _run(T, in_maps)
    out = np.zeros((B, H), np.float32)
    for c in range(NCORES):
        out[c * BL:(c + 1) * BL, :] = res.results[c]["out"].T
    return out


# revision 8
# speedup vs baseline: 1.2535x; 1.2535x over previous
"""Trainium2 Bass kernel for nn_AdaptiveRecurrentModel.

h_{t+1}[b,g] = tanh( (x_t @ Wi.T)[b,g] + sum_h h_t[b,h]*(W_base+ba_r+delta_t[b])[g,h]
                     + (bi+bias)[g] ),   delta_t[b] = (x_t @ Wa.T)[b].reshape(H,H)

Sharding: data-parallel over batch B (8 per core). Weights replicated.

Per-core layout strategy (no transposes anywhere in the T-step recurrence):
 - h kept as hT [h=128 partitions, b=8 free]; the step output tanh(...)[g, b]
   is directly the next step's input layout (g and h both index hidden dim).
 - delta precomputed in blocks of 16 steps (double-buffered) as matmul sweeps:
   for each g: delta[h, g*TB+tb] = WaT[:, g*H:(g+1)*H].T @ tokT_block.
 - per step, one PSUM tile [g, 8] accumulates: Wi matmul (N=8, no h dep,
   issues during previous tanh), W_base matmul (N=8), then 8 per-token
   matvecs with delta slices as stationary weights (N=1, column b).
   ScalarE applies tanh(psum + bias) -> next hT.
 - compute in bf16 (PE fp32 is dual-pass = 2x LDWEIGHTS+MATMUL; the system
   is fp32-chaotic anyway so trajectory-level precision is unattainable).
"""

import sys
from contextlib import ExitStack

import numpy as np

sys.path.insert(0, "/opt/trn_rl_repo")

import ml_dtypes  # noqa: E402
import concourse.bass as bass  # noqa: E402,F401
import concourse.tile as tile  # noqa: E402
from concourse import bacc, mybir  # noqa: E402
from concourse.bass_utils import run_bass_kernel_spmd  # noqa: E402

T_FULL, B, D, H = 256, 64, 128, 128
NCORES = 8
BL = B // NCORES  # 8 batch elements per core
F32 = mybir.dt.float32
BF16 = mybir.dt.bfloat16
BF16_NP = ml_dtypes.bfloat16

_nc_cache: dict = {}


def _make_nc(T: int):
    NTOK = T * BL
    TB = min(128, NTOK)           # tokens per delta block
    assert NTOK % TB == 0
    NBLK = NTOK // TB
    SPB = TB // BL                # steps per block (16)

    nc = bacc.Bacc("TRN2", target_bir_lowering=False, debug=False,
                   num_devices=NCORES)
    tokT_e = nc.dram_tensor("tokT", [D, NTOK], BF16, kind="ExternalInput").ap()
    waT_e = nc.dram_tensor("waT", [D, H * H], BF16, kind="ExternalInput").ap()
    wbT_e = nc.dram_tensor("wbT", [H, H], BF16, kind="ExternalInput").ap()
    wiT_e = nc.dram_tensor("wiT", [D, H], BF16, kind="ExternalInput").ap()
    biasv_e = nc.dram_tensor("biasv", [H, 1], F32, kind="ExternalInput").ap()
    out_e = nc.dram_tensor("out", [H, BL], F32, kind="ExternalOutput").ap()

    with tile.TileContext(nc) as tc:
        with ExitStack() as ctx:
            const = ctx.enter_context(tc.tile_pool(name="const", bufs=1))
            hpool = ctx.enter_context(tc.tile_pool(name="hstate", bufs=2))
            dpool = ctx.enter_context(tc.tile_pool(name="delta", bufs=2))
            pgen = ctx.enter_context(tc.tile_pool(name="pgen", bufs=3,
                                                  space="PSUM"))
            paccp = ctx.enter_context(tc.tile_pool(name="pacc", bufs=2,
                                                   space="PSUM"))

            # Wa^T split into 8 tiles so early delta matmuls can start while
            # later chunks are still streaming from HBM.
            NSPLIT = 8
            GPT = H // NSPLIT     # g's per wa tile (16)
            wa_tiles = []
            for i in range(NSPLIT):
                wt = const.tile([D, GPT * H], BF16, tag=f"wa{i}")
                nc.sync.dma_start(
                    wt[:], waT_e[:, i * GPT * H:(i + 1) * GPT * H])
                wa_tiles.append(wt)
            tokT = const.tile([D, NTOK], BF16, tag="tokT")
            nc.sync.dma_start(tokT[:], tokT_e[:])
            wbT = const.tile([H, H], BF16, tag="wbT")
            nc.sync.dma_start(wbT[:], wbT_e[:])
            wiT = const.tile([D, H], BF16, tag="wiT")
            nc.sync.dma_start(wiT[:], wiT_e[:])
            biasv = const.tile([H, 1], F32, tag="biasv")
            nc.sync.dma_start(biasv[:], biasv_e[:])

            hT = hpool.tile([H, BL], BF16, tag="h")
            nc.vector.memset(hT[:], 0.0)

            GG = 8                                 # g's per PSUM tile
            NCHUNK = H // GG                       # gen chunks per block (16)

            def gen_chunk(delta_t, kblk, g0):
                """Emit GG matmuls + one DVE add producing
                delta_t[:, g0*TB:(g0+GG)*TB] (with W_base^T folded in,
                broadcast over the token dim) for block kblk."""
                tok_blk = tokT[:, kblk * TB:(kblk + 1) * TB]
                ps = pgen.tile([H, GG * TB], F32, tag="pgen")
                for gg in range(GG):
                    g = g0 + gg
                    wt = wa_tiles[g // GPT]
                    lhsT = wt[:, (g % GPT) * H:((g % GPT) + 1) * H]
                    nc.tensor.matmul(ps[:, gg * TB:(gg + 1) * TB],
                                     lhsT, tok_blk,
                                     start=True, stop=True,
                                     skip_group_check=True)
                wb_b = (wbT[:, g0:g0 + GG].unsqueeze(2)
                        .broadcast_to([H, GG, TB]))
                dst = (delta_t[:, g0 * TB:(g0 + GG) * TB]
                       .rearrange("p (g t) -> p g t", t=TB))
                src = ps[:].rearrange("p (g t) -> p g t", t=TB)
                nc.vector.tensor_add(dst, src, wb_b)

            # prologue: block 0's delta
            cur = dpool.tile([H, H * TB], BF16, tag="delta")  # [h, g*TB+tb]
            for ci in range(NCHUNK):
                gen_chunk(cur, 0, ci * GG)

            for kblk in range(NBLK):
                if kblk + 1 < NBLK:
                    nxt = dpool.tile([H, H * TB], BF16, tag="delta")
                else:
                    nxt = None
                d_r = cur[:].rearrange("p (g t) -> p g t", t=TB)
                for s in range(SPB):
                    t = kblk * SPB + s
                    pacc = paccp.tile([H, BL], F32, tag="pacc")
                    nc.tensor.matmul(pacc[:, 0:BL], wiT[:],
                                     tokT[:, t * BL:(t + 1) * BL],
                                     start=True, stop=False,
                                     skip_group_check=True)
                    for b in range(BL):
                        tb = s * BL + b
                        nc.tensor.matmul(pacc[:, b:b + 1],
                                         d_r[:, :, tb:tb + 1],
                                         hT[:, b:b + 1],
                                         start=False, stop=(b == BL - 1),
                                         skip_group_check=True)
                    hT_new = hpool.tile([H, BL], BF16, tag="h")
                    nc.scalar.activation(hT_new[:], pacc[:],
                                         mybir.ActivationFunctionType.Tanh,
                                         bias=biasv[:])
                    hT = hT_new
                # next block's delta gen: contiguous dense matmul stretch
                if nxt is not None:
                    for ci in range(NCHUNK):
                        gen_chunk(nxt, kblk + 1, ci * GG)
                cur = nxt

            fin = const.tile([H, BL], F32, tag="fin")
            nc.vector.tensor_copy(fin[:], hT[:])
            nc.sync.dma_start(out_e[:], fin[:])
    nc.finalize()
    return nc


def _prep_inputs(tokens, Wi, bi, W_base, bias, Wa, ba):
    T = tokens.shape[0]
    Wb_eff = (np.asarray(W_base, np.float32)
              + np.asarray(ba, np.float32).reshape(H, H))
    waT = np.ascontiguousarray(np.asarray(Wa, np.float32).T
                               .astype(BF16_NP))                 # [d, g*H+h]
    wbT = np.ascontiguousarray(Wb_eff.T.astype(BF16_NP))         # [h, g]
    wiT = np.ascontiguousarray(np.asarray(Wi, np.float32).T
                               .astype(BF16_NP))                 # [d, g]
    biasv = np.ascontiguousarray(
        (np.asarray(bi, np.float32) + np.asarray(bias, np.float32))
        .reshape(H, 1))
    in_maps = []
    for c in range(NCORES):
        tok_c = np.asarray(tokens[:, c * BL:(c + 1) * BL, :], np.float32)
        tokT = np.ascontiguousarray(
            tok_c.transpose(2, 0, 1).reshape(D, T * BL).astype(BF16_NP))
        in_maps.append({"tokT": tokT, "waT": waT, "wbT": wbT,
                        "wiT": wiT, "biasv": biasv})
    return in_maps


def _run(T: int, in_maps, trace=False):
    if T not in _nc_cache:
        _nc_cache[T] = _make_nc(T)
    nc = _nc_cache[T]
    res = run_bass_kernel_spmd(nc, in_maps, core_ids=list(range(NCORES)),
                               trace=trace)
    return res


def kernel(tokens, Wi, bi, W_base, bias, Wa, ba):
    tokens = np.asarray(tokens, dtype=np.float32)
    T = tokens.shape[0]
    in_maps = _prep_inputs(tokens, Wi, bi, W_base, bias, Wa, ba)
    res = _run(T, in_maps)
    out = np.zeros((B, H), np.float32)
    for c in range(NCORES):
        out[c * BL:(c + 1) * BL, :] = res.results[c]["out"].T
    return out
